# revision 27
# baseline (speedup 1.0000x reference)
"""DeepseekV4-style attention (partial-RoPE LoRA-Q GQA sliding-window) on 8
Trainium2 NeuronCores.

Sharding: core c = 4*b + g handles batch b (of 2) and GQA group g (of 4):
q heads 4g..4g+3, kv head g, the matching column slices of Wq/Wk/Wv and row
slice of Wo.  Each core computes a partial output; the host sums the four
partials per batch.

v1 design (vs the fp32r baseline):
- All matmul operands bf16 (host-converted); PSUM accumulation stays f32.
- LoRA folded on host: Wq = Wqa @ Wqb[:, group] so the Q projection is a
  single GEMM and the duplicated qa stage disappears.
- Single-pass pipeline over 512-seq blocks: fused QKV projection -> rope ->
  attention (2 heads interleaved, PV one kt-step behind QK so the PE never
  waits on exp/mask) -> output projection, all within 8 PSUM banks.
- Softmax denominator: e-tiles accumulated on the Vector engine (esum += e),
  then ONE all-ones matmul per (head, block) instead of one per kt tile.
- reciprocal_approx_fast for 1/denominator.
"""

import numpy as np
import concourse.bass as bass
import concourse.mybir as mybir
import concourse.tile as tile
from concourse.bass_utils import run_bass_kernel_spmd

F32 = mybir.dt.float32
BF16 = mybir.dt.bfloat16
ACTF = mybir.ActivationFunctionType
ALU = mybir.AluOpType

B, S, D = 2, 2048, 2048
H, KVH, HD = 16, 4, 128
ROT, LORA, WINDOW = 64, 512, 1024
ROPE_BASE = 10000.0
SCALE = HD ** -0.5

INTERLEAVE_S4 = False   # emit stage4(sb-1) inside stage1(sb) waves

HPC = H // KVH          # 4 q heads per core
SB = 512                # free-dim block
NSB = S // SB           # 4 seq blocks
KT = D // 128           # 16 contraction tiles over D
ST = S // 128           # 16 seq 128-chunks
N_CORES = 8


def _split_multiwaits(nc):
    """This image's walrus accepts only one embedded SyncWait per instruction;
    split Tile's multi-wait sync_infos into standalone event-semaphore waits."""
    n = 0
    for func in nc.m.functions:
        for bb in func.blocks:
            insts = list(bb.instructions)
            out = []
            changed = False
            for inst in insts:
                si = inst.sync_info
                if si is not None and si.on_wait and len(si.on_wait) > 1:
                    waits = list(si.on_wait)
                    for w in waits[:-1]:
                        ev = mybir.InstEventSemaphore(
                            name=f"{inst.name}_wsplit_{n}", ins=[], outs=[]
                        )
                        ev.engine = inst.engine
                        ev.sync_info = mybir.SyncInfo(on_wait=[w], on_update=[])
                        out.append(ev)
                        n += 1
                    inst.sync_info = mybir.SyncInfo(
                        on_wait=[waits[-1]], on_update=list(si.on_update or [])
                    )
                    changed = True
                out.append(inst)
            if changed:
                bb.instructions = out
    return n


def build_nc(debug=False):
    nc = bass.Bass()
    hid = nc.dram_tensor("hid", [D, S], BF16, kind="ExternalInput")
    wq = nc.dram_tensor("wq", [D, HPC * HD], BF16, kind="ExternalInput")
    wkv = nc.dram_tensor("wkv", [D, 2 * HD], BF16, kind="ExternalInput")
    wo = nc.dram_tensor("wo", [HPC * HD, D], BF16, kind="ExternalInput")
    rcs = nc.dram_tensor("rcs", [128, S], BF16, kind="ExternalInput")
    out = nc.dram_tensor("out", [S, D], BF16, kind="ExternalOutput")
    if debug:
        qt_dbg = nc.dram_tensor("qt_dbg", [128, HPC * S], BF16, kind="ExternalOutput")
        kt_dbg = nc.dram_tensor("kt_dbg", [128, S], BF16, kind="ExternalOutput")
        vn_dbg = nc.dram_tensor("vn_dbg", [128, S], BF16, kind="ExternalOutput")
        at_dbg = nc.dram_tensor("at_dbg", [128, HPC * S], BF16, kind="ExternalOutput")

    with tile.TileContext(nc) as tc:
        with (
            tc.tile_pool(name="cst", bufs=1) as cst,
            tc.tile_pool(name="big", bufs=1) as big,
            tc.tile_pool(name="hp", bufs=2) as hp,
            tc.tile_pool(name="rp", bufs=2) as rp,
            tc.tile_pool(name="ex", bufs=4) as ex,
            tc.tile_pool(name="es", bufs=1) as es,
            tc.tile_pool(name="rcp", bufs=2) as rcp,
            tc.tile_pool(name="od", bufs=2) as od,
            tc.tile_pool(name="psP", bufs=1, space="PSUM") as psP,
            tc.tile_pool(name="psL", bufs=1, space="PSUM") as psL,
            tc.tile_pool(name="psO", bufs=1, space="PSUM") as psO,
        ):
            # ---- weights: per-kt tiles, DMAs interleaved for fast rampup ----
            wkv_t = [cst.tile([128, 256], BF16, tag=f"wkv{k}", name=f"wkv{k}") for k in range(KT)]
            wq_t = [cst.tile([128, 512], BF16, tag=f"wq{k}", name=f"wq{k}") for k in range(KT)]
            hb0 = hp.tile([128, KT * SB], BF16, tag="hb", name="hb_0")
            for k in range(KT):
                nc.sync.dma_start(out=wkv_t[k][:], in_=wkv[k * 128:(k + 1) * 128, :])
                nc.sync.dma_start(out=hb0[:, k * SB:(k + 1) * SB],
                                  in_=hid[k * 128:(k + 1) * 128, 0:SB])
                nc.sync.dma_start(out=wq_t[k][:], in_=wq[k * 128:(k + 1) * 128, :])

            # ---- constants ----
            ropeCC = cst.tile([64, S], BF16, tag="ropeCC")
            nc.sync.dma_start(out=ropeCC[:], in_=rcs[0:64, :])
            ropeSS = cst.tile([64, S], BF16, tag="ropeSS")
            nc.sync.dma_start(out=ropeSS[:], in_=rcs[64:128, :])
            onesf = cst.tile([128, 128], F32, tag="onesf")
            nc.vector.memset(onesf[:], 1.0)
            ones = cst.tile([128, 128], BF16, tag="ones")
            nc.vector.tensor_copy(ones[:], onesf[:])
            wo_t = [cst.tile([128, D], BF16, tag=f"wo{h}", name=f"wo{h}") for h in range(HPC)]
            for h in range(HPC):
                nc.sync.dma_start(out=wo_t[h][:], in_=wo[h * 128:(h + 1) * 128, :])

            # ---- persistent activations ----
            qT = big.tile([128, HPC * S], BF16, tag="qT")    # per-head Q^T [hd, s]
            kT = big.tile([128, S], BF16, tag="kT")
            vT = big.tile([128, S], BF16, tag="vT")
            vnat = big.tile([128, S], BF16, tag="vnat")      # V rows, chunk t at cols t*128

            def rope_apply(dst, sl, rsl):
                # dst rows 0:64 hold [x1; x2]; rotate in place (T-layout).
                # DVE ops need equal SBUF base partitions, so the half-swap
                # goes through a small SBUF->SBUF DMA.
                swp = rp.tile([64, SB], BF16, tag="swp")
                nc.sync.dma_start(out=swp[0:32, :], in_=dst[32:64, sl])
                nc.sync.dma_start(out=swp[32:64, :], in_=dst[0:32, sl])
                csb = rp.tile([64, SB], BF16, tag="csb")
                nc.vector.tensor_mul(csb[:], dst[0:64, sl], ropeCC[:, rsl])
                tsin = rp.tile([64, SB], BF16, tag="tsin")
                nc.vector.tensor_mul(tsin[:], swp[:], ropeSS[:, rsl])
                nc.vector.tensor_sub(dst[0:32, sl], csb[0:32, :], tsin[0:32, :])
                nc.vector.tensor_add(dst[32:64, sl], csb[32:64, :], tsin[32:64, :])

            # ---- stage 4 emitter: output projection for 128-row chunks ----
            attn_tiles = {}

            def emit_stage4(src_sb, t_list):
                attn_t = attn_tiles[src_sb]
                for t in t_list:
                    ti = t - src_sb * 4
                    ot = od.tile([128, D], BF16, tag="ot", name=f"ot_{t}")
                    for n in range(4):
                        pw = psL.tile([128, SB], F32, tag=f"pl{n % 2}",
                                      name=f"pw_{t}_{n}")
                        for h in range(HPC):
                            nc.tensor.matmul(
                                pw[:],
                                attn_t[:, h * SB + ti * 128:h * SB + (ti + 1) * 128],
                                wo_t[h][:, n * SB:(n + 1) * SB],
                                start=(h == 0), stop=(h == HPC - 1),
                            )
                        nc.vector.tensor_copy(ot[:, n * SB:(n + 1) * SB], pw[:])
                        if n == 1:
                            eng = nc.sync if t % 2 == 0 else nc.gpsimd
                            eng.dma_start(out=out[t * 128:(t + 1) * 128, 0:D // 2],
                                          in_=ot[:, 0:D // 2])
                        elif n == 3:
                            eng = nc.gpsimd if t % 2 == 0 else nc.sync
                            eng.dma_start(out=out[t * 128:(t + 1) * 128, D // 2:D],
                                          in_=ot[:, D // 2:D])

            # pipeline over seq blocks: proj(sb) -> rope -> attn(qb=sb) -> out(sb)
            # stage 4 of block sb-1 is interleaved between the projection waves
            # of block sb so wave-boundary PSUM stalls and the normalization
            # latency are hidden behind matmuls.
            hb_tiles = {0: hb0}
            for sb_i in range(NSB):
                sl = slice(sb_i * SB, (sb_i + 1) * SB)
                hb = hb_tiles[sb_i]

                # ---- stage 1: fused QKV projection, 3 waves of 2 psum banks ----
                # q heads first: their rope gates the first QK of this block's
                # attention; k/v last since the attention kt loop touches this
                # block's diagonal tiles only at its very end.
                waves = [
                    [("q", 0), ("q", 1)],
                    [("q", 2), ("q", 3)],
                    [("k", None), ("v", None)],
                ]
                for wi, wv in enumerate(waves):
                    pg = [psP.tile([128, SB], F32, tag=f"pg{i}",
                                   name=f"pg{i}_{sb_i}_{wv[0][0]}{wv[0][1]}")
                          for i in range(2)]
                    for k in range(KT):
                        for i, (kind, idx) in enumerate(wv):
                            if kind == "k":
                                w_ap = wkv_t[k][:, 0:128]
                            elif kind == "v":
                                w_ap = wkv_t[k][:, 128:256]
                            else:
                                w_ap = wq_t[k][:, idx * 128:(idx + 1) * 128]
                            nc.tensor.matmul(
                                pg[i][:], w_ap, hb[:, k * SB:(k + 1) * SB],
                                start=(k == 0), stop=(k == KT - 1),
                            )
                    for i, (kind, idx) in enumerate(wv):
                        if kind == "k":
                            nc.scalar.copy(kT[:, sl], pg[i][:])
                            rope_apply(kT, sl, sl)
                        elif kind == "v":
                            nc.scalar.copy(vT[:, sl], pg[i][:])
                        else:
                            dsl = slice(idx * S + sb_i * SB, idx * S + (sb_i + 1) * SB)
                            nc.scalar.copy(qT[:, dsl], pg[i][:])
                            rope_apply(qT, dsl, sl)
                    # fill the wave-boundary stall with prev block's stage 4
                    if INTERLEAVE_S4 and sb_i > 0 and wi < 2:
                        emit_stage4(sb_i - 1, [(sb_i - 1) * 4 + 2 * wi,
                                               (sb_i - 1) * 4 + 2 * wi + 1])

                # V natural: transpose the 4 128-chunks via the DMA xbar
                for t in range(sb_i * 4, sb_i * 4 + 4):
                    nc.sync.dma_start_transpose(
                        out=vnat[:, t * 128:(t + 1) * 128],
                        in_=vT[:, t * 128:(t + 1) * 128])

                # prefetch next block's hidden tiles (gpsimd queue: issues as
                # soon as the previous block's masking is done, well before
                # stage 1 of the next block needs them)
                if sb_i + 1 < NSB:
                    nsl = slice((sb_i + 1) * SB, (sb_i + 2) * SB)
                    nhb = hp.tile([128, KT * SB], BF16, tag="hb",
                                  name=f"hb_{sb_i + 1}")
                    hb_tiles[sb_i + 1] = nhb
                    for k in range(KT):
                        nc.gpsimd.dma_start(
                            out=nhb[:, k * SB:(k + 1) * SB],
                            in_=hid[k * 128:(k + 1) * 128, nsl])

                # ---- stage 3: attention for qb = sb_i, two heads interleaved ----
                q0 = sb_i * SB
                kt_lo = max(0, q0 - WINDOW + 1) // 128
                kt_hi = q0 // 128 + 3
                attn = rcp.tile([128, HPC * SB], BF16, tag="attn")
                attn_tiles[sb_i] = attn
                po_all = {}
                esum_all = {}

                def emit_norm(heads):
                    # denominator + 1/x + normalize for finished heads; pd
                    # borrows the projection psum banks (idle during attn)
                    for h in heads:
                        pd = psP.tile([128, SB], F32, tag=f"pg{h % 2}",
                                      name=f"pd_{sb_i}_{h}")
                        nc.tensor.matmul(pd[:], ones[:], esum_all[h][:],
                                         start=True, stop=True)
                        lnt = rcp.tile([128, SB], F32, tag="lnt")
                        nc.scalar.activation(lnt[:], pd[:], ACTF.Ln)
                        rec = rcp.tile([128, SB], F32, tag="rec")
                        nc.scalar.activation(rec[:], lnt[:], ACTF.Exp,
                                             scale=-1.0)
                        nc.vector.tensor_mul(attn[:, h * SB:(h + 1) * SB],
                                             po_all[h][:], rec[:])

                for pi, hp2 in enumerate(((0, 1), (2, 3))):
                    po = {}
                    esum = {}
                    e_cur = {}
                    for h in hp2:
                        po[h] = psO.tile([128, SB], F32, tag=f"po{h}",
                                         name=f"po{h}_{sb_i}")
                        esum[h] = es.tile([128, SB], BF16, tag=f"es{h}",
                                          name=f"es{h}_{sb_i}")
                        po_all[h] = po[h]
                        esum_all[h] = esum[h]
                    # software-pipelined: QK at kt, PV at kt-1
                    for step, kt in enumerate(range(kt_lo, kt_hi + 2)):
                        if pi == 1 and step == 2:
                            emit_norm((0, 1))
                        for j, h in enumerate(hp2):
                            if kt <= kt_hi:
                                qsl = slice(h * S + q0, h * S + q0 + SB)
                                dp = kt * 128 - q0
                                pl = psL.tile([128, SB], F32, tag=f"pl{j}",
                                              name=f"pl{j}_{sb_i}_{kt}")
                                nc.tensor.matmul(
                                    pl[:], kT[:, kt * 128:(kt + 1) * 128],
                                    qT[:, qsl], start=True, stop=True,
                                )
                                e = ex.tile([128, SB], BF16, tag="e")
                                nc.scalar.activation(e[:], pl[:], ACTF.Exp,
                                                     scale=SCALE)
                                if dp >= 0:
                                    # fills only hit columns j < 128+dp
                                    c1 = min(SB, 128 + dp)
                                    nc.gpsimd.affine_select(
                                        out=e[:, 0:c1], in_=e[:, 0:c1],
                                        pattern=[[1, c1]],
                                        compare_op=ALU.is_ge, fill=0.0,
                                        base=-dp, channel_multiplier=-1,
                                    )
                                elif dp <= -(WINDOW - SB + 128):
                                    # fills only hit columns j >= WINDOW+dp
                                    c0 = max(0, WINDOW + dp)
                                    nc.gpsimd.affine_select(
                                        out=e[:, c0:SB], in_=e[:, c0:SB],
                                        pattern=[[-1, SB - c0]],
                                        compare_op=ALU.is_ge, fill=0.0,
                                        base=WINDOW - 1 + dp - c0,
                                        channel_multiplier=1,
                                    )
                                if kt == kt_lo:
                                    nc.vector.tensor_copy(esum[h][:], e[:])
                                else:
                                    nc.vector.tensor_add(esum[h][:], esum[h][:],
                                                         e[:])
                                e_cur[h] = (kt, e)
                            if kt > kt_lo:
                                pkt, pe = e_prev[h]
                                nc.tensor.matmul(
                                    po[h][:], vnat[:, pkt * 128:(pkt + 1) * 128],
                                    pe[:], start=(pkt == kt_lo),
                                    stop=(pkt == kt_hi),
                                )
                        e_prev = dict(e_cur)
                emit_norm((2, 3))

                if not INTERLEAVE_S4:
                    emit_stage4(sb_i, list(range(sb_i * 4, sb_i * 4 + 4)))

                if debug:
                    for h in range(HPC):
                        nc.sync.dma_start(
                            out=at_dbg[:, h * S + sb_i * SB:h * S + (sb_i + 1) * SB],
                            in_=attn[:, h * SB:(h + 1) * SB])

            # drain: stage 4 of the last block
            if INTERLEAVE_S4:
                emit_stage4(NSB - 1, list(range((NSB - 1) * 4, NSB * 4)))

            if debug:
                nc.sync.dma_start(out=qt_dbg[:], in_=qT[:])
                nc.sync.dma_start(out=kt_dbg[:], in_=kT[:])
                nc.sync.dma_start(out=vn_dbg[:], in_=vnat[:])
    _split_multiwaits(nc)
    return nc


_NC = None


def _get_nc():
    global _NC
    if _NC is None:
        _NC = build_nc()
    return _NC


def _make_in_maps(hidden, position_ids, Wqa, Wqb, Wk, Wv, Wo):
    import ml_dtypes
    bf16 = ml_dtypes.bfloat16
    hidden = np.asarray(hidden, dtype=np.float32)
    position_ids = np.asarray(position_ids)
    Wqa = np.asarray(Wqa, dtype=np.float32)
    Wqb = np.asarray(Wqb, dtype=np.float32)
    Wk = np.asarray(Wk, dtype=np.float32)
    Wv = np.asarray(Wv, dtype=np.float32)
    Wo = np.asarray(Wo, dtype=np.float32)

    inv_freq = 1.0 / (ROPE_BASE ** (np.arange(0, ROT, 2, dtype=np.float32) / ROT))
    hidT = [np.ascontiguousarray(hidden[b].T).astype(bf16) for b in range(B)]
    Wq_full = Wqa @ Wqb  # [D, H*HD] folded LoRA
    in_maps = []
    for c in range(N_CORES):
        b, g = c // KVH, c % KVH
        pos = position_ids[b].astype(np.float32)
        freqs = pos[:, None] * inv_freq[None, :]        # [S, 32]
        cosT = np.cos(freqs).T.astype(np.float32)       # [32, S]
        sinT = np.sin(freqs).T.astype(np.float32)
        rcs = np.concatenate([cosT, cosT, sinT, sinT], axis=0)  # [128, S]
        in_maps.append({
            "hid": hidT[b],
            "wq": np.ascontiguousarray(
                Wq_full[:, g * HPC * HD:(g + 1) * HPC * HD]).astype(bf16),
            "wkv": np.ascontiguousarray(
                np.concatenate(
                    [Wk[:, g * HD:(g + 1) * HD], Wv[:, g * HD:(g + 1) * HD]], axis=1
                )).astype(bf16),
            "wo": np.ascontiguousarray(
                Wo[g * HPC * HD:(g + 1) * HPC * HD, :]).astype(bf16),
            "rcs": np.ascontiguousarray(rcs).astype(bf16),
        })
    return in_maps


def _run(inputs, trace=False):
    nc = _get_nc()
    in_maps = _make_in_maps(**inputs)
    res = run_bass_kernel_spmd(nc, in_maps, list(range(N_CORES)), trace=trace)
    out = np.zeros((B, S, D), dtype=np.float32)
    for c in range(N_CORES):
        out[c // KVH] += res.results[c]["out"].astype(np.float32)
    return out, res


def kernel(**inputs) -> np.ndarray:
    return _run(inputs, trace=False)[0]


# revision 31
# speedup vs baseline: 1.0000x; 1.0000x over previous
"""DeepseekV4-style attention (partial-RoPE LoRA-Q GQA sliding-window) on 8
Trainium2 NeuronCores.

Sharding: core c = 4*b + g handles batch b (of 2) and GQA group g (of 4):
q heads 4g..4g+3, kv head g, the matching column slices of Wq/Wk/Wv and row
slice of Wo.  Each core computes a partial output; the host sums the four
partials per batch.

v1 design (vs the fp32r baseline):
- All matmul operands bf16 (host-converted); PSUM accumulation stays f32.
- LoRA folded on host: Wq = Wqa @ Wqb[:, group] so the Q projection is a
  single GEMM and the duplicated qa stage disappears.
- Single-pass pipeline over 512-seq blocks: fused QKV projection -> rope ->
  attention (2 heads interleaved, PV one kt-step behind QK so the PE never
  waits on exp/mask) -> output projection, all within 8 PSUM banks.
- Softmax denominator: e-tiles accumulated on the Vector engine (esum += e),
  then ONE all-ones matmul per (head, block) instead of one per kt tile.
- reciprocal_approx_fast for 1/denominator.
"""

import numpy as np
import concourse.bass as bass
import concourse.mybir as mybir
import concourse.tile as tile
from concourse.bass_utils import run_bass_kernel_spmd

F32 = mybir.dt.float32
BF16 = mybir.dt.bfloat16
ACTF = mybir.ActivationFunctionType
ALU = mybir.AluOpType

B, S, D = 2, 2048, 2048
H, KVH, HD = 16, 4, 128
ROT, LORA, WINDOW = 64, 512, 1024
ROPE_BASE = 10000.0
SCALE = HD ** -0.5

INTERLEAVE_S4 = False   # emit stage4(sb-1) inside stage1(sb) waves

HPC = H // KVH          # 4 q heads per core
SB = 512                # free-dim block
NSB = S // SB           # 4 seq blocks
KT = D // 128           # 16 contraction tiles over D
ST = S // 128           # 16 seq 128-chunks
N_CORES = 8


def _split_multiwaits(nc):
    """This image's walrus accepts only one embedded SyncWait per instruction;
    split Tile's multi-wait sync_infos into standalone event-semaphore waits."""
    n = 0
    for func in nc.m.functions:
        for bb in func.blocks:
            insts = list(bb.instructions)
            out = []
            changed = False
            for inst in insts:
                si = inst.sync_info
                if si is not None and si.on_wait and len(si.on_wait) > 1:
                    waits = list(si.on_wait)
                    for w in waits[:-1]:
                        ev = mybir.InstEventSemaphore(
                            name=f"{inst.name}_wsplit_{n}", ins=[], outs=[]
                        )
                        ev.engine = inst.engine
                        ev.sync_info = mybir.SyncInfo(on_wait=[w], on_update=[])
                        out.append(ev)
                        n += 1
                    inst.sync_info = mybir.SyncInfo(
                        on_wait=[waits[-1]], on_update=list(si.on_update or [])
                    )
                    changed = True
                out.append(inst)
            if changed:
                bb.instructions = out
    return n


def build_nc(debug=False):
    nc = bass.Bass()
    hid = nc.dram_tensor("hid", [D, S], BF16, kind="ExternalInput")
    wq = nc.dram_tensor("wq", [D, HPC * HD], BF16, kind="ExternalInput")
    wkv = nc.dram_tensor("wkv", [D, 2 * HD], BF16, kind="ExternalInput")
    wo = nc.dram_tensor("wo", [HPC * HD, D], BF16, kind="ExternalInput")
    rcs = nc.dram_tensor("rcs", [128, S], BF16, kind="ExternalInput")
    out = nc.dram_tensor("out", [S, D], BF16, kind="ExternalOutput")
    if debug:
        qt_dbg = nc.dram_tensor("qt_dbg", [128, HPC * S], BF16, kind="ExternalOutput")
        kt_dbg = nc.dram_tensor("kt_dbg", [128, S], BF16, kind="ExternalOutput")
        vn_dbg = nc.dram_tensor("vn_dbg", [128, S], BF16, kind="ExternalOutput")
        at_dbg = nc.dram_tensor("at_dbg", [128, HPC * S], BF16, kind="ExternalOutput")

    with tile.TileContext(nc) as tc:
        with (
            tc.tile_pool(name="cst", bufs=1) as cst,
            tc.tile_pool(name="big", bufs=1) as big,
            tc.tile_pool(name="hp", bufs=2) as hp,
            tc.tile_pool(name="rp", bufs=2) as rp,
            tc.tile_pool(name="ex", bufs=4) as ex,
            tc.tile_pool(name="es", bufs=1) as es,
            tc.tile_pool(name="rcp", bufs=2) as rcp,
            tc.tile_pool(name="od", bufs=2) as od,
            tc.tile_pool(name="psP", bufs=1, space="PSUM") as psP,
            tc.tile_pool(name="psL", bufs=1, space="PSUM") as psL,
            tc.tile_pool(name="psO", bufs=1, space="PSUM") as psO,
        ):
            # ---- weights: per-kt tiles, DMAs interleaved for fast rampup ----
            wkv_t = [cst.tile([128, 256], BF16, tag=f"wkv{k}", name=f"wkv{k}") for k in range(KT)]
            wq_t = [cst.tile([128, 512], BF16, tag=f"wq{k}", name=f"wq{k}") for k in range(KT)]
            hb0 = hp.tile([128, KT * SB], BF16, tag="hb", name="hb_0")
            for k in range(KT):
                nc.sync.dma_start(out=wkv_t[k][:], in_=wkv[k * 128:(k + 1) * 128, :])
                nc.sync.dma_start(out=hb0[:, k * SB:(k + 1) * SB],
                                  in_=hid[k * 128:(k + 1) * 128, 0:SB])
                nc.sync.dma_start(out=wq_t[k][:], in_=wq[k * 128:(k + 1) * 128, :])

            # ---- constants ----
            ropeCC = cst.tile([64, S], BF16, tag="ropeCC")
            nc.sync.dma_start(out=ropeCC[:], in_=rcs[0:64, :])
            ropeSS = cst.tile([64, S], BF16, tag="ropeSS")
            nc.sync.dma_start(out=ropeSS[:], in_=rcs[64:128, :])
            onesf = cst.tile([128, 128], F32, tag="onesf")
            nc.vector.memset(onesf[:], 1.0)
            ones = cst.tile([128, 128], BF16, tag="ones")
            nc.vector.tensor_copy(ones[:], onesf[:])
            wo_t = [cst.tile([128, D], BF16, tag=f"wo{h}", name=f"wo{h}") for h in range(HPC)]
            for h in range(HPC):
                nc.sync.dma_start(out=wo_t[h][:], in_=wo[h * 128:(h + 1) * 128, :])

            # ---- persistent activations ----
            qT = big.tile([128, HPC * S], BF16, tag="qT")    # per-head Q^T [hd, s]
            kT = big.tile([128, S], BF16, tag="kT")
            vT = big.tile([128, S], BF16, tag="vT")
            # V natural chunks as separate tiles: the xbar-transpose DMA's
            # write dependency is tracked per-tile, so per-chunk tiles keep
            # PV matmuls from waiting on this block's transposes.
            vn_t = [big.tile([128, 128], BF16, tag=f"vn{t}", name=f"vn{t}")
                    for t in range(ST)]

            def rope_apply(dst, sl, rsl):
                # dst rows 0:64 hold [x1; x2]; rotate in place (T-layout).
                # DVE ops need equal SBUF base partitions, so the half-swap
                # goes through a small SBUF->SBUF DMA.
                swp = rp.tile([64, SB], BF16, tag="swp")
                nc.sync.dma_start(out=swp[0:32, :], in_=dst[32:64, sl])
                nc.sync.dma_start(out=swp[32:64, :], in_=dst[0:32, sl])
                csb = rp.tile([64, SB], BF16, tag="csb")
                nc.vector.tensor_mul(csb[:], dst[0:64, sl], ropeCC[:, rsl])
                tsin = rp.tile([64, SB], BF16, tag="tsin")
                nc.vector.tensor_mul(tsin[:], swp[:], ropeSS[:, rsl])
                nc.vector.tensor_sub(dst[0:32, sl], csb[0:32, :], tsin[0:32, :])
                nc.vector.tensor_add(dst[32:64, sl], csb[32:64, :], tsin[32:64, :])

            # ---- stage 4 emitter: output projection for 128-row chunks ----
            attn_tiles = {}

            def emit_stage4(src_sb, t_list):
                attn_t = attn_tiles[src_sb]
                for t in t_list:
                    ti = t - src_sb * 4
                    ot = od.tile([128, D], BF16, tag="ot", name=f"ot_{t}")
                    for n in range(4):
                        pw = psL.tile([128, SB], F32, tag=f"pl{n % 2}",
                                      name=f"pw_{t}_{n}")
                        for h in range(HPC):
                            nc.tensor.matmul(
                                pw[:],
                                attn_t[:, h * SB + ti * 128:h * SB + (ti + 1) * 128],
                                wo_t[h][:, n * SB:(n + 1) * SB],
                                start=(h == 0), stop=(h == HPC - 1),
                            )
                        nc.vector.tensor_copy(ot[:, n * SB:(n + 1) * SB], pw[:])
                        if n == 1:
                            eng = nc.sync if t % 2 == 0 else nc.gpsimd
                            eng.dma_start(out=out[t * 128:(t + 1) * 128, 0:D // 2],
                                          in_=ot[:, 0:D // 2])
                        elif n == 3:
                            eng = nc.gpsimd if t % 2 == 0 else nc.sync
                            eng.dma_start(out=out[t * 128:(t + 1) * 128, D // 2:D],
                                          in_=ot[:, D // 2:D])

            # pipeline over seq blocks: proj(sb) -> rope -> attn(qb=sb) -> out(sb)
            # stage 4 of block sb-1 is interleaved between the projection waves
            # of block sb so wave-boundary PSUM stalls and the normalization
            # latency are hidden behind matmuls.
            hb_tiles = {0: hb0}
            for sb_i in range(NSB):
                sl = slice(sb_i * SB, (sb_i + 1) * SB)
                hb = hb_tiles[sb_i]

                # ---- stage 1: fused QKV projection, 3 waves of 2 psum banks ----
                # q heads first: their rope gates the first QK of this block's
                # attention; k/v last since the attention kt loop touches this
                # block's diagonal tiles only at its very end.
                waves = [
                    [("q", 0), ("q", 1)],
                    [("q", 2), ("q", 3)],
                    [("k", None), ("v", None)],
                ]
                for wi, wv in enumerate(waves):
                    pg = [psP.tile([128, SB], F32, tag=f"pg{i}",
                                   name=f"pg{i}_{sb_i}_{wv[0][0]}{wv[0][1]}")
                          for i in range(2)]
                    for k in range(KT):
                        for i, (kind, idx) in enumerate(wv):
                            if kind == "k":
                                w_ap = wkv_t[k][:, 0:128]
                            elif kind == "v":
                                w_ap = wkv_t[k][:, 128:256]
                            else:
                                w_ap = wq_t[k][:, idx * 128:(idx + 1) * 128]
                            nc.tensor.matmul(
                                pg[i][:], w_ap, hb[:, k * SB:(k + 1) * SB],
                                start=(k == 0), stop=(k == KT - 1),
                            )
                    for i, (kind, idx) in enumerate(wv):
                        if kind == "k":
                            nc.scalar.copy(kT[:, sl], pg[i][:])
                            rope_apply(kT, sl, sl)
                        elif kind == "v":
                            nc.scalar.copy(vT[:, sl], pg[i][:])
                        else:
                            dsl = slice(idx * S + sb_i * SB, idx * S + (sb_i + 1) * SB)
                            nc.scalar.copy(qT[:, dsl], pg[i][:])
                            rope_apply(qT, dsl, sl)
                    # fill the wave-boundary stall with prev block's stage 4
                    if INTERLEAVE_S4 and sb_i > 0 and wi < 2:
                        emit_stage4(sb_i - 1, [(sb_i - 1) * 4 + 2 * wi,
                                               (sb_i - 1) * 4 + 2 * wi + 1])

                # V natural: transpose the 4 128-chunks via the DMA xbar
                for t in range(sb_i * 4, sb_i * 4 + 4):
                    nc.sync.dma_start_transpose(
                        out=vn_t[t][:],
                        in_=vT[:, t * 128:(t + 1) * 128])

                # prefetch next block's hidden tiles (gpsimd queue: issues as
                # soon as the previous block's masking is done, well before
                # stage 1 of the next block needs them)
                if sb_i + 1 < NSB:
                    nsl = slice((sb_i + 1) * SB, (sb_i + 2) * SB)
                    nhb = hp.tile([128, KT * SB], BF16, tag="hb",
                                  name=f"hb_{sb_i + 1}")
                    hb_tiles[sb_i + 1] = nhb
                    for k in range(KT):
                        nc.gpsimd.dma_start(
                            out=nhb[:, k * SB:(k + 1) * SB],
                            in_=hid[k * 128:(k + 1) * 128, nsl])

                # ---- stage 3: attention for qb = sb_i, two heads interleaved ----
                q0 = sb_i * SB
                kt_lo = max(0, q0 - WINDOW + 1) // 128
                kt_hi = q0 // 128 + 3
                attn = rcp.tile([128, HPC * SB], BF16, tag="attn")
                attn_tiles[sb_i] = attn
                po_all = {}
                esum_all = {}

                def emit_norm(heads):
                    # denominator + 1/x + normalize for finished heads; pd
                    # borrows the projection psum banks (idle during attn)
                    for h in heads:
                        pd = psP.tile([128, SB], F32, tag=f"pg{h % 2}",
                                      name=f"pd_{sb_i}_{h}")
                        nc.tensor.matmul(pd[:], ones[:], esum_all[h][:],
                                         start=True, stop=True)
                        lnt = rcp.tile([128, SB], F32, tag="lnt")
                        nc.scalar.activation(lnt[:], pd[:], ACTF.Ln)
                        rec = rcp.tile([128, SB], F32, tag="rec")
                        nc.scalar.activation(rec[:], lnt[:], ACTF.Exp,
                                             scale=-1.0)
                        nc.vector.tensor_mul(attn[:, h * SB:(h + 1) * SB],
                                             po_all[h][:], rec[:])

                for pi, hp2 in enumerate(((0, 1), (2, 3))):
                    po = {}
                    esum = {}
                    e_cur = {}
                    for h in hp2:
                        po[h] = psO.tile([128, SB], F32, tag=f"po{h}",
                                         name=f"po{h}_{sb_i}")
                        esum[h] = es.tile([128, SB], BF16, tag=f"es{h}",
                                          name=f"es{h}_{sb_i}")
                        po_all[h] = po[h]
                        esum_all[h] = esum[h]
                    # software-pipelined: QK at kt, PV at kt-1
                    for step, kt in enumerate(range(kt_lo, kt_hi + 2)):
                        if pi == 1 and step == 2:
                            emit_norm((0, 1))
                        for j, h in enumerate(hp2):
                            if kt <= kt_hi:
                                qsl = slice(h * S + q0, h * S + q0 + SB)
                                dp = kt * 128 - q0
                                pl = psL.tile([128, SB], F32, tag=f"pl{j}",
                                              name=f"pl{j}_{sb_i}_{kt}")
                                nc.tensor.matmul(
                                    pl[:], kT[:, kt * 128:(kt + 1) * 128],
                                    qT[:, qsl], start=True, stop=True,
                                )
                                e = ex.tile([128, SB], BF16, tag="e")
                                nc.scalar.activation(e[:], pl[:], ACTF.Exp,
                                                     scale=SCALE)
                                if dp >= 0:
                                    # fills only hit columns j < 128+dp
                                    c1 = min(SB, 128 + dp)
                                    nc.gpsimd.affine_select(
                                        out=e[:, 0:c1], in_=e[:, 0:c1],
                                        pattern=[[1, c1]],
                                        compare_op=ALU.is_ge, fill=0.0,
                                        base=-dp, channel_multiplier=-1,
                                    )
                                elif dp <= -(WINDOW - SB + 128):
                                    # fills only hit columns j >= WINDOW+dp
                                    c0 = max(0, WINDOW + dp)
                                    nc.gpsimd.affine_select(
                                        out=e[:, c0:SB], in_=e[:, c0:SB],
                                        pattern=[[-1, SB - c0]],
                                        compare_op=ALU.is_ge, fill=0.0,
                                        base=WINDOW - 1 + dp - c0,
                                        channel_multiplier=1,
                                    )
                                if kt == kt_lo:
                                    nc.vector.tensor_copy(esum[h][:], e[:])
                                else:
                                    nc.vector.tensor_add(esum[h][:], esum[h][:],
                                                         e[:])
                                e_cur[h] = (kt, e)
                            if kt > kt_lo:
                                pkt, pe = e_prev[h]
                                nc.tensor.matmul(
                                    po[h][:], vn_t[pkt][:],
                                    pe[:], start=(pkt == kt_lo),
                                    stop=(pkt == kt_hi),
                                )
                        e_prev = dict(e_cur)
                emit_norm((2, 3))

                if not INTERLEAVE_S4:
                    emit_stage4(sb_i, list(range(sb_i * 4, sb_i * 4 + 4)))

                if debug:
                    for h in range(HPC):
                        nc.sync.dma_start(
                            out=at_dbg[:, h * S + sb_i * SB:h * S + (sb_i + 1) * SB],
                            in_=attn[:, h * SB:(h + 1) * SB])

            # drain: stage 4 of the last block
            if INTERLEAVE_S4:
                emit_stage4(NSB - 1, list(range((NSB - 1) * 4, NSB * 4)))

            if debug:
                nc.sync.dma_start(out=qt_dbg[:], in_=qT[:])
                nc.sync.dma_start(out=kt_dbg[:], in_=kT[:])
                for t in range(ST):
                    nc.sync.dma_start(out=vn_dbg[:, t * 128:(t + 1) * 128],
                                      in_=vn_t[t][:])
    _split_multiwaits(nc)
    return nc


_NC = None


def _get_nc():
    global _NC
    if _NC is None:
        _NC = build_nc()
    return _NC


def _make_in_maps(hidden, position_ids, Wqa, Wqb, Wk, Wv, Wo):
    import ml_dtypes
    bf16 = ml_dtypes.bfloat16
    hidden = np.asarray(hidden, dtype=np.float32)
    position_ids = np.asarray(position_ids)
    Wqa = np.asarray(Wqa, dtype=np.float32)
    Wqb = np.asarray(Wqb, dtype=np.float32)
    Wk = np.asarray(Wk, dtype=np.float32)
    Wv = np.asarray(Wv, dtype=np.float32)
    Wo = np.asarray(Wo, dtype=np.float32)

    inv_freq = 1.0 / (ROPE_BASE ** (np.arange(0, ROT, 2, dtype=np.float32) / ROT))
    hidT = [np.ascontiguousarray(hidden[b].T).astype(bf16) for b in range(B)]
    Wq_full = Wqa @ Wqb  # [D, H*HD] folded LoRA
    in_maps = []
    for c in range(N_CORES):
        b, g = c // KVH, c % KVH
        pos = position_ids[b].astype(np.float32)
        freqs = pos[:, None] * inv_freq[None, :]        # [S, 32]
        cosT = np.cos(freqs).T.astype(np.float32)       # [32, S]
        sinT = np.sin(freqs).T.astype(np.float32)
        rcs = np.concatenate([cosT, cosT, sinT, sinT], axis=0)  # [128, S]
        in_maps.append({
            "hid": hidT[b],
            "wq": np.ascontiguousarray(
                Wq_full[:, g * HPC * HD:(g + 1) * HPC * HD]).astype(bf16),
            "wkv": np.ascontiguousarray(
                np.concatenate(
                    [Wk[:, g * HD:(g + 1) * HD], Wv[:, g * HD:(g + 1) * HD]], axis=1
                )).astype(bf16),
            "wo": np.ascontiguousarray(
                Wo[g * HPC * HD:(g + 1) * HPC * HD, :]).astype(bf16),
            "rcs": np.ascontiguousarray(rcs).astype(bf16),
        })
    return in_maps


def _run(inputs, trace=False):
    nc = _get_nc()
    in_maps = _make_in_maps(**inputs)
    res = run_bass_kernel_spmd(nc, in_maps, list(range(N_CORES)), trace=trace)
    out = np.zeros((B, S, D), dtype=np.float32)
    for c in range(N_CORES):
        out[c // KVH] += res.results[c]["out"].astype(np.float32)
    return out, res


def kernel(**inputs) -> np.ndarray:
    return _run(inputs, trace=False)[0]
